# revision 1
# baseline (speedup 1.0000x reference)
"""Tensor-parallel GQA attention layer for 8 Trainium2 NeuronCores.

Shapes (hardcoded from the problem spec):
  x [1, 2048, 4096] f32, wq [4096, 4096], wk/wv [1024, 4096],
  wo [4096, 4096], freqs_cos/sin [2048, 64], mask [2048, 2048].

Sharding: tensor-parallel over heads. Core i owns q-heads 4i..4i+3 and
kv-head i (wq/wk/wv column-parallel). The output projection is sharded
over OUTPUT columns instead of rows: head outputs are AllGathered
(2MB/core) and each core computes out[:, 512i:512(i+1)], avoiding the
32MB all-reduce a row-parallel wo would need.

Numerics: matmuls in bf16 (fp32 PSUM accumulation); softmax in fp32 on
the scalar engine without max-subtraction (scores are O(1) by
construction); masking via elementwise multiply with exp(mask), applied
only to tiles where exp(mask) is neither all-ones nor all-zero
(all-zero tiles are skipped entirely, which for the causal mask removes
~38% of attention work).
"""

import math
import sys

for _p in ("/opt/trn_rl_repo",):
    if _p not in sys.path:
        sys.path.append(_p)

import numpy as np
import ml_dtypes

import concourse.bass as bass
import concourse.mybir as mybir
import concourse.tile as tile
from concourse.bass_utils import run_bass_kernel_spmd
from concourse.masks import make_identity
from concourse.vector_clock import ScopedClock

BF16 = mybir.dt.bfloat16
F32 = mybir.dt.float32
AF = mybir.ActivationFunctionType

N_CORES = 8
DIM = 4096
SEQ = 2048
HD = 128                      # head dim == partition dim
NQH = 4                       # q heads per core
P = 128
SC = 512                      # seq chunk (psum bank free size in f32)
ND = DIM // P                 # 32 contraction tiles
NSC = SEQ // SC               # 4 seq chunks
NKT = SEQ // P                # 16 k tiles
QCOLS = NQH * HD              # 512 q columns per core

LAST_RESULT = None            # BassKernelResults of the most recent kernel() call


def _patch_tile_drain():
    """The walrus build in this container rejects Drain instructions that
    carry more than one sync-wait (and sem-eq waits). Spread the tile-exit
    waits across single-wait nops and use sem-only barriers instead."""

    def patched(self, tick_clock, wait_clock):
        carrier = self.nc.sync.nop(nofuse=True)
        wait_clock.add_sem_waits(
            carrier.ins, ScopedClock({None: tick_clock.global_clock})
        )
        si = carrier.ins.sync_info
        waits = list(si.on_wait) if si and si.on_wait else []
        if len(waits) > 1:
            si.on_wait = waits[:1]
            for w in waits[1:]:
                extra = self.nc.sync.nop(nofuse=True)
                extra.ins.sync_info = mybir.SyncInfo(on_wait=[w], on_update=[])
        self.nc.sync.drain()
        self.nc.all_engine_barrier(sem_only=True)
        popped = self.nc._tile_sem_poison_stack.pop()
        assert popped is self._sem_poison
        self.nc.clear_and_free_semaphores(list(self.sems.allocated().values()))
        self.nc.all_engine_barrier(sem_only=True)

    tile.TileContext._drain_and_barrier = patched


_patch_tile_drain()


def _split_multi_waits(nc, limit=1):
    """This walrus build supports ~one sync-wait per instruction (and none
    on Drain). Hoist excess waits onto single-wait NoOps inserted just
    before the instruction on the same engine queue (FIFO => equivalent)."""
    for fn in nc.m.functions:
        for bb in fn.blocks:
            out = []
            changed = False
            for ins in bb.instructions:
                si = getattr(ins, "sync_info", None)
                waits = list(si.on_wait) if si is not None and si.on_wait else []
                keep = 0 if type(ins).__name__ == "InstDrain" else limit
                if len(waits) > keep:
                    changed = True
                    for w in waits[keep:]:
                        nop = mybir.InstNoOp(
                            name=f"WSPLIT-{nc.next_id()}", ins=[], outs=[])
                        nop.engine = ins.engine
                        nop.sync_info = mybir.SyncInfo(on_wait=[w], on_update=[])
                        out.append(nop)
                    si.on_wait = waits[:keep]
                out.append(ins)
            if changed:
                bb.instructions[:] = out


def _classify_mask(mask):
    """Per (ki, qj) tile classes of exp(mask).T: 0=no-op, 1=multiply, 2=skip.

    Returns (classes [NKT, NSC], packed mixed tiles [n_mixed*P, SC] bf16,
    mixed index map {(ki, qj): packed_idx}).
    """
    em = np.exp(mask.astype(np.float64)).astype(np.float32).T  # [k, q]
    classes = np.zeros((NKT, NSC), dtype=np.int32)
    mixed = []
    mixed_idx = {}
    for ki in range(NKT):
        for qj in range(NSC):
            t = em[ki * P:(ki + 1) * P, qj * SC:(qj + 1) * SC]
            if np.all(t == 1.0):
                classes[ki, qj] = 0
            elif np.all(t == 0.0):
                classes[ki, qj] = 2
            else:
                classes[ki, qj] = 1
                mixed_idx[(ki, qj)] = len(mixed)
                mixed.append(t.astype(ml_dtypes.bfloat16))
    if mixed:
        packed = np.concatenate(mixed, axis=0)
    else:
        packed = np.zeros((P, SC), dtype=ml_dtypes.bfloat16)
    return classes, packed, mixed_idx


def _build_program(classes, mixed_idx, n_mixed):
    nc = bass.Bass()

    xT_d = nc.dram_tensor("xT", [DIM, SEQ], BF16, kind="ExternalInput")
    wqT_d = nc.dram_tensor("wqT", [DIM, QCOLS], BF16, kind="ExternalInput")
    wkT_d = nc.dram_tensor("wkT", [DIM, HD], BF16, kind="ExternalInput")
    wvT_d = nc.dram_tensor("wvT", [DIM, HD], BF16, kind="ExternalInput")
    woT_d = nc.dram_tensor("woT", [DIM, QCOLS], BF16, kind="ExternalInput")
    ropeC_d = nc.dram_tensor("ropeC", [P, SEQ], F32, kind="ExternalInput")
    ropeS_d = nc.dram_tensor("ropeS", [P, SEQ], F32, kind="ExternalInput")
    mm_d = nc.dram_tensor("maskmul", [max(n_mixed, 1) * P, SC], BF16,
                          kind="ExternalInput")
    out_d = nc.dram_tensor("out", [SEQ, QCOLS], F32, kind="ExternalOutput")

    scale = 1.0 / math.sqrt(HD)
    H = P // 2

    with tile.TileContext(nc) as tc:
        with tc.tile_pool(name="const", bufs=1) as cp, \
             tc.tile_pool(name="acts", bufs=1) as ap, \
             tc.tile_pool(name="wo", bufs=1) as wop, \
             tc.tile_pool(name="mask", bufs=1) as mmp:
            ident = cp.tile([P, P], BF16, tag="ident", name="ident")
            make_identity(nc, ident[:])
            ones_col = cp.tile([P, 1], BF16, tag="ones_col", name="ones_col")
            nc.gpsimd.memset(ones_col[:], 1.0)
            ones_row = cp.tile([1, P], F32, tag="ones_row", name="ones_row")
            nc.gpsimd.memset(ones_row[:], 1.0)

            qT = [ap.tile([P, SEQ], BF16, tag=f"qT{h}", name=f"qT{h}")
                  for h in range(NQH)]
            kT = ap.tile([P, SEQ], BF16, tag="kT", name="kT")
            V = [ap.tile([P, HD], BF16, tag=f"V{t}", name=f"V{t}")
                 for t in range(NKT)]
            attnT = [ap.tile([P, SEQ], BF16, tag=f"attnT{h}", name=f"attnT{h}")
                     for h in range(NQH)]

            wo_sb = [wop.tile([P, QCOLS], BF16, tag=f"wo{c}", name=f"wo{c}")
                     for c in range(ND)]
            for c in range(ND):
                nc.sync.dma_start(wo_sb[c][:], woT_d[c * P:(c + 1) * P, :])
            mask_sb = [mmp.tile([P, SC], BF16, tag=f"mm{i}", name=f"mm{i}")
                       for i in range(max(n_mixed, 1))]
            for i in range(n_mixed):
                nc.sync.dma_start(mask_sb[i][:], mm_d[i * P:(i + 1) * P, :])
            dp = tc.alloc_tile_pool(name="dram", bufs=1, space="DRAM")
            cc_in = [dp.tile([NQH * P, SC], BF16, tag=f"cc_in{w}",
                             name=f"cc_in{w}") for w in range(NSC)]
            cc_out = [dp.tile([N_CORES * NQH * P, SC], BF16, tag=f"cc_out{w}",
                              name=f"cc_out{w}", addr_space="Shared")
                      for w in range(NSC)]

            def rope_apply(rp, src, dst, ssl, ropeC, ropeS):
                tsw = rp.tile([P, SC], F32, tag="tsw", name="tsw")
                nc.scalar.activation(tsw[0:H, :], src[H:P, :], AF.Copy)
                nc.scalar.activation(tsw[H:P, :], src[0:H, :], AF.Copy)
                t1 = rp.tile([P, SC], F32, tag="t1", name="t1")
                nc.vector.tensor_mul(t1[:], src[:], ropeC[:, ssl])
                t2 = rp.tile([P, SC], F32, tag="t2", name="t2")
                nc.vector.tensor_mul(t2[:], tsw[:], ropeS[:, ssl])
                nc.vector.tensor_add(dst[:, ssl], t1[:], t2[:])

            def attention(h, qj, pools):
                pssp, psump, pavp, pbp, ep, r2p = pools
                if True:
                    qsl = slice(qj * SC, (qj + 1) * SC)
                    live = [ki for ki in range(NKT) if classes[ki, qj] != 2]
                    pairs = [live[i:i + 2] for i in range(0, len(live), 2)]
                    Es = []  # (ki, e_tile, col_slice)
                    for pi, pair in enumerate(pairs):
                        n = len(pair)
                        pss = pssp.tile([P, 2 * SC], F32, tag="pss", name="pss")
                        for x, ki in enumerate(pair):
                            nc.tensor.matmul(
                                pss[:, x * SC:(x + 1) * SC],
                                kT[:, ki * P:(ki + 1) * P],
                                qT[h][:, qsl], start=True, stop=True)
                        e = ep.tile([P, 2 * SC], BF16, tag=f"E{pi}", name=f"E{pi}")
                        nc.scalar.activation(e[:, 0:n * SC], pss[:, 0:n * SC],
                                             AF.Exp, scale=scale)
                        for x, ki in enumerate(pair):
                            esl = slice(x * SC, (x + 1) * SC)
                            if classes[ki, qj] == 1:
                                nc.vector.tensor_mul(
                                    e[:, esl], e[:, esl],
                                    mask_sb[mixed_idx[(ki, qj)]][:])
                            Es.append((ki, e, esl))
                    psum = psump.tile([1, SC], F32, tag="psum", name="psum")
                    for i, (ki, e, esl) in enumerate(Es):
                        nc.tensor.matmul(psum[:], ones_col[:], e[:, esl],
                                         start=i == 0, stop=i == len(Es) - 1)
                    pav = pavp.tile([P, SC], F32, tag="pav", name="pav")
                    for i, (ki, e, esl) in enumerate(Es):
                        nc.tensor.matmul(pav[:], V[ki][:], e[:, esl],
                                         start=i == 0, stop=i == len(Es) - 1)
                    sums = r2p.tile([1, SC], F32, tag="sums", name="sums")
                    nc.vector.tensor_copy(sums[:], psum[:])
                    pb = pbp.tile([P, SC], F32, tag="pb", name="pb")
                    nc.tensor.matmul(pb[:], ones_row[:], sums[:],
                                     start=True, stop=True)
                    bsb = r2p.tile([P, SC], F32, tag="bsb", name="bsb")
                    nc.vector.reciprocal(bsb[:], pb[:])
                    nc.vector.tensor_mul(attnT[h][:, qsl], pav[:], bsb[:])

            with tc.tile_pool(name="E", bufs=1) as ep, \
                 tc.tile_pool(name="r2", bufs=2) as r2p, \
                 tc.tile_pool(name="w1", bufs=1) as wp, \
                 tc.tile_pool(name="xtA", bufs=3) as xpA, \
                 tc.tile_pool(name="rtA", bufs=1) as rpA:
                ropeC = wp.tile([P, SEQ], F32, tag="ropeC", name="ropeC")
                nc.sync.dma_start(ropeC[:], ropeC_d[:])
                ropeS = wp.tile([P, SEQ], F32, tag="ropeS", name="ropeS")
                nc.sync.dma_start(ropeS[:], ropeS_d[:])
                wq_sb = [wp.tile([P, QCOLS], BF16, tag=f"wq{d}", name=f"wq{d}")
                         for d in range(ND)]
                for d in range(ND):
                    nc.sync.dma_start(wq_sb[d][:], wqT_d[d * P:(d + 1) * P, :])

                # ---- kv pass: full K/V projection + RoPE(k) + V transpose ----
                with tc.tile_pool(name="wkv", bufs=1) as wkvp, \
                     tc.tile_pool(name="pskv", bufs=1, space="PSUM") as pskv, \
                     tc.tile_pool(name="ptr", bufs=1, space="PSUM") as ptrp:
                    wk_sb = [wkvp.tile([P, HD], BF16, tag=f"wk{d}", name=f"wk{d}")
                             for d in range(ND)]
                    wv_sb = [wkvp.tile([P, HD], BF16, tag=f"wv{d}", name=f"wv{d}")
                             for d in range(ND)]
                    for d in range(ND):
                        nc.sync.dma_start(wk_sb[d][:], wkT_d[d * P:(d + 1) * P, :])
                        nc.sync.dma_start(wv_sb[d][:], wvT_d[d * P:(d + 1) * P, :])
                    W2 = 2 * SC
                    for bc in range(SEQ // W2):
                        bsl = slice(bc * W2, (bc + 1) * W2)
                        psk = pskv.tile([P, W2], F32, tag="psk", name="psk")
                        psv = pskv.tile([P, W2], F32, tag="psv", name="psv")
                        for d in range(ND):
                            xt = xpA.tile([P, W2], BF16, tag="xt", name="xt")
                            nc.sync.dma_start(xt[:], xT_d[d * P:(d + 1) * P, bsl])
                            st, sp = d == 0, d == ND - 1
                            for x in range(2):
                                xsl = slice(x * SC, (x + 1) * SC)
                                nc.tensor.matmul(psk[:, xsl], wk_sb[d][:],
                                                 xt[:, xsl], start=st, stop=sp)
                                nc.tensor.matmul(psv[:, xsl], wv_sb[d][:],
                                                 xt[:, xsl], start=st, stop=sp)
                        for x in range(2):
                            xsl = slice(x * SC, (x + 1) * SC)
                            ssl = slice(bc * W2 + x * SC, bc * W2 + (x + 1) * SC)
                            rope_apply(rpA, psk[:, xsl], kT, ssl, ropeC, ropeS)
                            vtmp = rpA.tile([P, SC], BF16, tag="vtmp", name="vtmp")
                            nc.scalar.activation(vtmp[:], psv[:, xsl], AF.Copy)
                            for t in range(SC // P):
                                ptr = ptrp.tile([P, P], BF16, tag="ptr", name="ptr")
                                nc.tensor.transpose(
                                    ptr[:], vtmp[:, t * P:(t + 1) * P], ident[:])
                                nc.scalar.activation(
                                    V[(bc * W2 + x * SC) // P + t][:], ptr[:],
                                    AF.Copy)

                # ---- per-q-chunk blocks, largest (most causal work) first ----
                for qj in range(NSC - 1, -1, -1):
                    ssl = slice(qj * SC, (qj + 1) * SC)
                    with tc.tile_pool(name="pq", bufs=1, space="PSUM") as pqp:
                        psq = [pqp.tile([P, SC], F32, tag=f"psq{h}",
                                        name=f"psq{h}") for h in range(NQH)]
                        for d in range(ND):
                            xt = xpA.tile([P, SC], BF16, tag="xtq", name="xtq")
                            nc.sync.dma_start(xt[:], xT_d[d * P:(d + 1) * P, ssl])
                            st, sp = d == 0, d == ND - 1
                            for h in range(NQH):
                                nc.tensor.matmul(
                                    psq[h][:], wq_sb[d][:, h * HD:(h + 1) * HD],
                                    xt[:], start=st, stop=sp)
                        for h in range(NQH):
                            rope_apply(rpA, psq[h], qT[h], ssl, ropeC, ropeS)

                    with tc.tile_pool(name="ps1", bufs=2, space="PSUM") as pssp, \
                         tc.tile_pool(name="ps1s", bufs=1, space="PSUM") as psump, \
                         tc.tile_pool(name="ps1a", bufs=2, space="PSUM") as pavp, \
                         tc.tile_pool(name="ps1b", bufs=1, space="PSUM") as pbp:
                        pools = (pssp, psump, pavp, pbp, ep, r2p)
                        for h in range(NQH):
                            attention(h, qj, pools)
                            nc.gpsimd.dma_start(
                                cc_in[qj][h * P:(h + 1) * P, :],
                                attnT[h][:, ssl])
                        nc.gpsimd.collective_compute(
                            "AllGather", mybir.AluOpType.bypass,
                            replica_groups=[list(range(N_CORES))],
                            ins=[cc_in[qj].opt()], outs=[cc_out[qj].opt()])

            # ---- phase 3: output projection columns ----
            # cc_out[g] row-tile r -> core r//2, head 2g + r%2
            #   => global c-tile 4*(r//2) + 2g + r%2
            with tc.tile_pool(name="ah", bufs=1) as ahp, \
                 tc.tile_pool(name="po", bufs=2, space="PSUM") as pop, \
                 tc.tile_pool(name="ob", bufs=2) as obp:
                NS4 = SC // P
                NR = N_CORES * NQH
                for w in range(NSC - 1, -1, -1):
                    ah = {}
                    for r in range(NR):
                        c = r  # core r//NQH, local head r%NQH == global c-tile r
                        ah[c] = ahp.tile([P, SC], BF16, tag=f"ah{c}", name=f"ah{c}")
                        nc.sync.dma_start(
                            ah[c][:], cc_out[w][r * P:(r + 1) * P, :])
                    po = [pop.tile([P, QCOLS], F32, tag=f"po{s4}", name=f"po{s4}")
                          for s4 in range(NS4)]
                    for c in range(NR):
                        for s4 in range(NS4):
                            nc.tensor.matmul(
                                po[s4][:], ah[c][:, s4 * P:(s4 + 1) * P],
                                wo_sb[c][:], start=c == 0, stop=c == NR - 1)
                    for s4 in range(NS4):
                        st = w * NS4 + s4
                        ob = obp.tile([P, QCOLS], F32, tag="ob", name="ob")
                        nc.scalar.activation(ob[:], po[s4][:], AF.Copy)
                        nc.sync.dma_start(out_d[st * P:(st + 1) * P, :], ob[:])
            dp.release()

    _split_multi_waits(nc)
    return nc


def kernel(x, wq, wk, wv, wo, freqs_cos, freqs_sin, mask):
    x = np.asarray(x, dtype=np.float32)
    wq = np.asarray(wq, dtype=np.float32)
    wk = np.asarray(wk, dtype=np.float32)
    wv = np.asarray(wv, dtype=np.float32)
    wo = np.asarray(wo, dtype=np.float32)
    freqs_cos = np.asarray(freqs_cos, dtype=np.float32)
    freqs_sin = np.asarray(freqs_sin, dtype=np.float32)
    mask = np.asarray(mask, dtype=np.float32)

    bf = ml_dtypes.bfloat16
    # deinterleave head_dim pairs so RoPE becomes a partition-half swap
    perm = np.concatenate([np.arange(0, HD, 2), np.arange(1, HD, 2)])
    wq_p = wq.reshape(-1, HD, DIM)[:, perm, :].reshape(wq.shape)
    wk_p = wk.reshape(-1, HD, DIM)[:, perm, :].reshape(wk.shape)

    xT = np.ascontiguousarray(x[0].T).astype(bf)               # [DIM, SEQ]
    ropeC = np.ascontiguousarray(
        np.concatenate([freqs_cos.T, freqs_cos.T], axis=0))     # [128, SEQ]
    ropeS = np.ascontiguousarray(
        np.concatenate([-freqs_sin.T, freqs_sin.T], axis=0))

    classes, maskpack, mixed_idx = _classify_mask(mask)
    n_mixed = len(mixed_idx)

    nc = _build_program(classes, mixed_idx, n_mixed)

    in_maps = []
    for i in range(N_CORES):
        wqT = np.ascontiguousarray(
            wq_p[i * QCOLS:(i + 1) * QCOLS, :].T).astype(bf)    # [DIM, 512]
        wkT = np.ascontiguousarray(
            wk_p[i * HD:(i + 1) * HD, :].T).astype(bf)          # [DIM, 128]
        wvT = np.ascontiguousarray(
            wv[i * HD:(i + 1) * HD, :].T).astype(bf)
        # out[:, 512i:512(i+1)] = attn_full @ wo.T[:, 512i:...]
        woT = np.ascontiguousarray(
            wo[i * QCOLS:(i + 1) * QCOLS, :].T).astype(bf)      # [DIM, 512]
        in_maps.append({
            "xT": xT, "wqT": wqT, "wkT": wkT, "wvT": wvT, "woT": woT,
            "ropeC": ropeC, "ropeS": ropeS, "maskmul": maskpack,
        })

    res = run_bass_kernel_spmd(nc, in_maps, list(range(N_CORES)))
    global LAST_RESULT
    LAST_RESULT = res
    out = np.concatenate(
        [np.asarray(res.results[i]["out"]) for i in range(N_CORES)], axis=1)
    return out.reshape(1, SEQ, DIM).astype(np.float32)



# revision 9
# speedup vs baseline: 1.4906x; 1.4906x over previous
"""Tensor-parallel GQA attention layer for 8 Trainium2 NeuronCores.

Shapes (hardcoded from the problem spec):
  x [1, 2048, 4096] f32, wq [4096, 4096], wk/wv [1024, 4096],
  wo [4096, 4096], freqs_cos/sin [2048, 64], mask [2048, 2048].

Sharding: tensor-parallel over heads. Core i owns q-heads 4i..4i+3 and
kv-head i (wq/wk/wv column-parallel). Head outputs are AllGathered per
512-seq chunk (512KB/core) and each core computes out[:, 512i:512(i+1)].

v2 structure (vs v1): single fused QKV pass over x (x streamed once),
static PSUM bank map so no pool-boundary barriers, softmax sums via
vector-chain adds + one f32r matmul per block, reciprocal_approx_fast,
out-projection interleaved chunk-by-chunk behind the AllGathers, DMA
issue in consumption order split between the sync and gpsimd queues.
"""

import math
import sys

for _p in ("/opt/trn_rl_repo",):
    if _p not in sys.path:
        sys.path.append(_p)

import numpy as np
import ml_dtypes

import concourse.bass as bass
import concourse.mybir as mybir
import concourse.tile as tile
from concourse.bass_utils import run_bass_kernel_spmd
from concourse.masks import make_identity
from concourse.vector_clock import ScopedClock

BF16 = mybir.dt.bfloat16
F32 = mybir.dt.float32
F32R = mybir.dt.float32r
AF = mybir.ActivationFunctionType

N_CORES = 8
DIM = 4096
SEQ = 2048
HD = 128                      # head dim == partition dim
NQH = 4                       # q heads per core
P = 128
SC = 512                      # seq chunk (psum bank free size in f32)
ND = DIM // P                 # 32 contraction tiles
NSC = SEQ // SC               # 4 seq chunks
NKT = SEQ // P                # 16 k tiles
QCOLS = NQH * HD              # 512 q columns per core
H = P // 2

LAST_RESULT = None            # BassKernelResults of the most recent kernel() call


def _patch_tile_drain():
    """The walrus build in this container rejects Drain instructions that
    carry more than one sync-wait (and sem-eq waits). Spread the tile-exit
    waits across single-wait nops and use sem-only barriers instead."""

    def patched(self, tick_clock, wait_clock):
        carrier = self.nc.sync.nop(nofuse=True)
        wait_clock.add_sem_waits(
            carrier.ins, ScopedClock({None: tick_clock.global_clock})
        )
        si = carrier.ins.sync_info
        waits = list(si.on_wait) if si and si.on_wait else []
        if len(waits) > 1:
            si.on_wait = waits[:1]
            for w in waits[1:]:
                extra = self.nc.sync.nop(nofuse=True)
                extra.ins.sync_info = mybir.SyncInfo(on_wait=[w], on_update=[])
        self.nc.sync.drain()
        self.nc.all_engine_barrier(sem_only=True)
        popped = self.nc._tile_sem_poison_stack.pop()
        assert popped is self._sem_poison
        self.nc.clear_and_free_semaphores(list(self.sems.allocated().values()))
        self.nc.all_engine_barrier(sem_only=True)

    tile.TileContext._drain_and_barrier = patched


_patch_tile_drain()


def _split_multi_waits(nc, limit=1):
    """This walrus build supports ~one sync-wait per instruction (and none
    on Drain). Hoist excess waits onto single-wait NoOps inserted just
    before the instruction on the same engine queue (FIFO => equivalent)."""
    for fn in nc.m.functions:
        for bb in fn.blocks:
            out = []
            changed = False
            for ins in bb.instructions:
                si = getattr(ins, "sync_info", None)
                waits = list(si.on_wait) if si is not None and si.on_wait else []
                keep = 0 if type(ins).__name__ == "InstDrain" else limit
                if len(waits) > keep:
                    changed = True
                    for w in waits[keep:]:
                        nop = mybir.InstNoOp(
                            name=f"WSPLIT-{nc.next_id()}", ins=[], outs=[])
                        nop.engine = ins.engine
                        nop.sync_info = mybir.SyncInfo(on_wait=[w], on_update=[])
                        out.append(nop)
                    si.on_wait = waits[:keep]
                out.append(ins)
            if changed:
                bb.instructions[:] = out


def _classify_mask(mask):
    """Per (ki, qj) tile classes of exp(mask).T: 0=no-op, 1=multiply, 2=skip.

    Returns (classes [NKT, NSC], packed unique mixed tiles [128, n_u*SC]
    bf16, map {(ki, qj): unique_idx}).
    """
    em = np.exp(mask.astype(np.float64)).astype(np.float32).T  # [k, q]
    classes = np.zeros((NKT, NSC), dtype=np.int32)
    uniq = []
    uniq_keys = {}
    mixed_map = {}
    for ki in range(NKT):
        for qj in range(NSC):
            t = em[ki * P:(ki + 1) * P, qj * SC:(qj + 1) * SC]
            if np.all(t == 1.0):
                classes[ki, qj] = 0
            elif np.all(t == 0.0):
                classes[ki, qj] = 2
            else:
                classes[ki, qj] = 1
                tb = t.astype(ml_dtypes.bfloat16)
                key = tb.tobytes()
                if key not in uniq_keys:
                    uniq_keys[key] = len(uniq)
                    uniq.append(tb)
                mixed_map[(ki, qj)] = uniq_keys[key]
    if uniq:
        packed = np.concatenate(uniq, axis=1)       # [128, n_u*SC]
    else:
        packed = np.zeros((P, SC), dtype=ml_dtypes.bfloat16)
    return classes, packed, mixed_map


def _build_program(classes, mixed_map, n_u):
    nc = bass.Bass()

    xT_d = nc.dram_tensor("xT", [DIM, SEQ], BF16, kind="ExternalInput")
    wkv_d = nc.dram_tensor("wkvP", [P, 2 * ND * HD], BF16, kind="ExternalInput")
    wq_d = nc.dram_tensor("wqP", [P, ND * QCOLS], BF16, kind="ExternalInput")
    wo_d = nc.dram_tensor("woP", [P, ND * QCOLS], BF16, kind="ExternalInput")
    ropeC_d = nc.dram_tensor("ropeC", [P, SEQ], BF16, kind="ExternalInput")
    ropeS_d = nc.dram_tensor("ropeS", [P, SEQ], BF16, kind="ExternalInput")
    mm_d = nc.dram_tensor("maskmul", [P, max(n_u, 1) * SC], BF16,
                          kind="ExternalInput")
    out_d = nc.dram_tensor("out", [SEQ, QCOLS], F32, kind="ExternalOutput")

    scale = 1.0 / math.sqrt(HD)

    with tile.TileContext(nc) as tc, \
         tc.tile_pool(name="const", bufs=1) as cp, \
         tc.tile_pool(name="weights", bufs=1) as wp, \
         tc.tile_pool(name="acts", bufs=1) as ap, \
         tc.tile_pool(name="xw", bufs=5) as xp, \
         tc.tile_pool(name="ropet", bufs=1) as rp, \
         tc.tile_pool(name="esoft", bufs=1) as ep, \
         tc.tile_pool(name="outb", bufs=1) as op, \
         tc.tile_pool(name="ps", bufs=1, space="PSUM") as psp:
        dp = tc.alloc_tile_pool(name="dram", bufs=1, space="DRAM")

        # ---- constants ----
        ident = cp.tile([P, P], BF16, tag="ident", name="ident")
        make_identity(nc, ident[:])
        ones_col = cp.tile([P, 1], BF16, tag="ones_col", name="ones_col")
        nc.gpsimd.memset(ones_col[:], 1.0)
        ones_row = cp.tile([1, P], BF16, tag="ones_row", name="ones_row")
        nc.gpsimd.memset(ones_row[:], 1.0)

        # ---- persistent SBUF tensors ----
        wkv_sb = wp.tile([P, 2 * ND * HD], BF16, tag="wkv", name="wkv_sb")
        wq_sb = wp.tile([P, ND * QCOLS], BF16, tag="wq", name="wq_sb")
        ropeC = wp.tile([P, SEQ], BF16, tag="ropeC", name="ropeC")
        ropeS = wp.tile([P, SEQ], BF16, tag="ropeS", name="ropeS")
        mask_sb = wp.tile([P, max(n_u, 1) * SC], BF16, tag="mm", name="mask_sb")

        kT = ap.tile([P, SEQ], BF16, tag="kT", name="kT")
        qT = [ap.tile([P, SEQ], BF16, tag=f"qT{h}", name=f"qT{h}")
              for h in range(NQH)]
        V_sb = ap.tile([P, NKT * HD], BF16, tag="V", name="V_sb")
        attnT = [ap.tile([P, SEQ], BF16, tag=f"attnT{h}", name=f"attnT{h}")
                 for h in range(NQH)]

        # ---- DMA issue: weights on sync queue, consumption order ----
        for i in range(8):
            w = 2 * ND * HD // 8
            nc.sync.dma_start(wkv_sb[:, i * w:(i + 1) * w],
                              wkv_d[:, i * w:(i + 1) * w])
        for i in range(4):
            w = SEQ // 4
            nc.sync.dma_start(ropeC[:, i * w:(i + 1) * w],
                              ropeC_d[:, i * w:(i + 1) * w])
            nc.sync.dma_start(ropeS[:, i * w:(i + 1) * w],
                              ropeS_d[:, i * w:(i + 1) * w])
        for i in range(8):
            w = ND * QCOLS // 8
            nc.sync.dma_start(wq_sb[:, i * w:(i + 1) * w],
                              wq_d[:, i * w:(i + 1) * w])
        nc.sync.dma_start(mask_sb[:], mm_d[:])

        # shared DRAM scratch for the collectives
        cc_in = [dp.tile([NQH * P, SC], BF16, tag=f"cc_in{w}",
                         name=f"cc_in{w}") for w in range(NSC)]
        cc_out = [dp.tile([N_CORES * NQH * P, SC], BF16, tag=f"cc_out{w}",
                          name=f"cc_out{w}", addr_space="Shared")
                  for w in range(NSC)]

        # PSUM: 8 static one-bank tags; phase assignments keep overlapping
        # phases on disjoint tags so nothing ever waits on a pool boundary.
        def ps_tile(i, shape, name, dtype=F32):
            return psp.tile(shape, dtype, tag=f"ps{i}", name=name)

        def rope_apply(src, dst, ssl):
            """src: [128, SC] f32 PSUM in deinterleaved-pair layout; writes
            dst[:, ssl] (bf16 SBUF). ropeC=[c;c], ropeS=[-s;s]."""
            tsw = rp.tile([P, SC], F32, tag="tsw", name="tsw")
            nc.scalar.activation(tsw[0:H, :], src[H:P, :], AF.Copy)
            nc.scalar.activation(tsw[H:P, :], src[0:H, :], AF.Copy)
            t1 = rp.tile([P, SC], F32, tag="t1", name="t1")
            nc.vector.tensor_mul(t1[:], src[:], ropeC[:, ssl])
            t2 = rp.tile([P, SC], F32, tag="t2", name="t2")
            nc.vector.tensor_mul(t2[:], tsw[:], ropeS[:, ssl])
            nc.vector.tensor_add(dst[:, ssl], t1[:], t2[:])

        # ---- fused QKV pass: one stream over x ----
        for c in range(NSC):
            ssl = slice(c * SC, (c + 1) * SC)
            xts = []
            for do in range(4):
                xt8 = xp.tile([P, 8 * SC], BF16, tag="xt8", name="xt8")
                for j in range(8):
                    d = do * 8 + j
                    nc.gpsimd.dma_start(
                        xt8[:, j * SC:(j + 1) * SC],
                        xT_d[d * P:(d + 1) * P, ssl])
                xts.append(xt8)
            psk = ps_tile(0, [P, SC], "psk")
            psv = ps_tile(1, [P, SC], "psv")
            psq = [ps_tile(2 + h, [P, SC], f"psq{h}") for h in range(NQH)]
            for d in range(ND):
                xsl = xts[d // 8][:, (d % 8) * SC:(d % 8 + 1) * SC]
                st, sp = d == 0, d == ND - 1
                nc.tensor.matmul(psk[:], wkv_sb[:, d * HD:(d + 1) * HD],
                                 xsl, start=st, stop=sp)
                nc.tensor.matmul(
                    psv[:], wkv_sb[:, ND * HD + d * HD:ND * HD + (d + 1) * HD],
                    xsl, start=st, stop=sp)
            for d in range(ND):
                xsl = xts[d // 8][:, (d % 8) * SC:(d % 8 + 1) * SC]
                st, sp = d == 0, d == ND - 1
                for h in range(NQH):
                    nc.tensor.matmul(
                        psq[h][:],
                        wq_sb[:, d * QCOLS + h * HD:d * QCOLS + (h + 1) * HD],
                        xsl, start=st, stop=sp)
            # evacuations
            rope_apply(psk, kT, ssl)
            vtmp = rp.tile([P, SC], BF16, tag="vtmp", name="vtmp", bufs=2)
            nc.scalar.activation(vtmp[:], psv[:], AF.Copy)
            for h in range(NQH):
                rope_apply(psq[h], qT[h], ssl)
            # V transposes for this chunk (banks 6/7 are free during QKV)
            for t in range(SC // P):
                kt = c * (SC // P) + t
                ptr = ps_tile(6 + t % 2, [P, P], "ptr", dtype=BF16)
                nc.tensor.transpose(ptr[:], vtmp[:, t * P:(t + 1) * P],
                                    ident[:])
                nc.scalar.activation(V_sb[:, kt * HD:(kt + 1) * HD], ptr,
                                     AF.Copy)

        # wo loads reuse the x-stream slots (x fully consumed by then)
        wo_t = []
        for i in range(4):
            wt = xp.tile([P, 8 * SC], BF16, tag="xt8", name="wo_t")
            for j in range(2):
                w = 4 * SC
                nc.sync.dma_start(
                    wt[:, j * w:(j + 1) * w],
                    wo_d[:, i * 8 * SC + j * w:i * 8 * SC + (j + 1) * w])
            wo_t.append(wt)

        # ---- attention + allgather + out-projection, pipelined ----
        blk = [0]

        def attn_block(h, qj):
            qsl = slice(qj * SC, (qj + 1) * SC)
            live = [ki for ki in range(NKT) if classes[ki, qj] != 2]
            L = len(live)
            pav = ps_tile(2 + blk[0] % 2, [P, SC], "pav")
            blk[0] += 1
            esum = ep.tile([P, SC], F32, tag="esum", name="esum", bufs=2)
            for i, ki in enumerate(live):
                pss = ps_tile(i % 2, [P, SC], "pss")
                nc.tensor.matmul(pss[:], kT[:, ki * P:(ki + 1) * P],
                                 qT[h][:, qsl], start=True, stop=True)
                e = ep.tile([P, SC], BF16, tag="e", name="e", bufs=5)
                nc.scalar.activation(e[:], pss[:], AF.Exp, scale=scale)
                u = mixed_map.get((ki, qj))
                if u is not None:
                    nc.vector.tensor_mul(e[:], e[:],
                                         mask_sb[:, u * SC:(u + 1) * SC])
                nc.tensor.matmul(pav[:], V_sb[:, ki * HD:(ki + 1) * HD],
                                 e[:], start=i == 0, stop=i == L - 1)
                if i == 0:
                    nc.vector.tensor_copy(esum[:], e[:])
                else:
                    nc.vector.tensor_add(esum[:], esum[:], e[:])
            esb = ep.tile([P, SC], BF16, tag="esb", name="esb", bufs=2)
            nc.vector.tensor_copy(esb[:], esum[:])
            sums = ps_tile(4, [1, SC], "sums")
            nc.tensor.matmul(sums[:], ones_col[:], esb[:],
                             start=True, stop=True)
            # 1/s = exp(-ln s) on the scalar engine: [1,512] ops are
            # per-lane serial, and DVE reciprocal costs ~4us there.
            lns = ep.tile([1, SC], F32, tag="rs", name="lns", bufs=2)
            nc.scalar.activation(lns[:], sums[:], AF.Ln)
            rs16 = ep.tile([1, SC], BF16, tag="rs16", name="rs16", bufs=2)
            nc.scalar.activation(rs16[:], lns[:], AF.Exp, scale=-1.0)
            pb = ps_tile(5, [P, SC], "pb")
            nc.tensor.matmul(pb[:], ones_row[:], rs16[:],
                             start=True, stop=True)
            bsb = ep.tile([P, SC], BF16, tag="bsb", name="bsb", bufs=2)
            nc.scalar.activation(bsb[:], pb[:], AF.Copy)
            nc.vector.tensor_mul(attnT[h][:, qsl], pav[:], bsb[:])

        def allgather(qj):
            qsl = slice(qj * SC, (qj + 1) * SC)
            for h in range(NQH):
                nc.gpsimd.dma_start(cc_in[qj][h * P:(h + 1) * P, :],
                                    attnT[h][:, qsl])
            nc.gpsimd.collective_compute(
                "AllGather", mybir.AluOpType.bypass,
                replica_groups=[list(range(N_CORES))],
                ins=[cc_in[qj].opt()], outs=[cc_out[qj].opt()])

        def outproj(w):
            NR = N_CORES * NQH
            ahs = []
            for r in range(NR):
                ah = op.tile([P, SC], BF16, tag="ah", name="ah", bufs=34)
                nc.sync.dma_start(ah[:], cc_out[w][r * P:(r + 1) * P, :])
                ahs.append(ah)
            for s4 in range(SC // P):
                po = ps_tile(6 + s4 % 2, [P, QCOLS], "po")
                for c in range(NR):
                    nc.tensor.matmul(
                        po[:], ahs[c][:, s4 * P:(s4 + 1) * P],
                        wo_t[c // 8][:, (c % 8) * SC:(c % 8 + 1) * SC],
                        start=c == 0, stop=c == NR - 1)
                ob = op.tile([P, QCOLS], F32, tag="ob", name="ob", bufs=2)
                nc.scalar.activation(ob[:], po[:], AF.Copy)
                st = w * (SC // P) + s4
                nc.sync.dma_start(out_d[st * P:(st + 1) * P, :], ob[:])

        # ascending: smallest chunk first; out-proj w fills the next
        # chunk's softmax stalls and hides the AllGather latency.
        for h in range(NQH):
            attn_block(h, 0)
        allgather(0)
        for h in range(NQH):
            attn_block(h, 1)
        allgather(1)
        outproj(0)
        for h in range(NQH):
            attn_block(h, 2)
        allgather(2)
        outproj(1)
        for h in range(NQH):
            attn_block(h, 3)
        allgather(3)
        outproj(2)
        outproj(3)
        dp.release()

    _split_multi_waits(nc)
    return nc


def _pack_dmajor(wT):
    """[DIM, W] -> [128, ND*W] with d-tile d at cols [d*W, (d+1)*W)."""
    w = wT.shape[1]
    return np.ascontiguousarray(
        wT.reshape(ND, P, w).transpose(1, 0, 2).reshape(P, ND * w))


def kernel(x, wq, wk, wv, wo, freqs_cos, freqs_sin, mask):
    x = np.asarray(x, dtype=np.float32)
    wq = np.asarray(wq, dtype=np.float32)
    wk = np.asarray(wk, dtype=np.float32)
    wv = np.asarray(wv, dtype=np.float32)
    wo = np.asarray(wo, dtype=np.float32)
    freqs_cos = np.asarray(freqs_cos, dtype=np.float32)
    freqs_sin = np.asarray(freqs_sin, dtype=np.float32)
    mask = np.asarray(mask, dtype=np.float32)

    bf = ml_dtypes.bfloat16
    # deinterleave head_dim pairs so RoPE becomes a partition-half swap
    perm = np.concatenate([np.arange(0, HD, 2), np.arange(1, HD, 2)])
    wq_p = wq.reshape(-1, HD, DIM)[:, perm, :].reshape(wq.shape)
    wk_p = wk.reshape(-1, HD, DIM)[:, perm, :].reshape(wk.shape)

    xT = np.ascontiguousarray(x[0].T).astype(bf)               # [DIM, SEQ]
    ropeC = np.ascontiguousarray(
        np.concatenate([freqs_cos.T, freqs_cos.T], axis=0)).astype(bf)
    ropeS = np.ascontiguousarray(
        np.concatenate([-freqs_sin.T, freqs_sin.T], axis=0)).astype(bf)

    classes, maskpack, mixed_map = _classify_mask(mask)
    n_u = maskpack.shape[1] // SC if mixed_map else 0

    nc = _build_program(classes, mixed_map, n_u)

    in_maps = []
    for i in range(N_CORES):
        wqT = np.ascontiguousarray(
            wq_p[i * QCOLS:(i + 1) * QCOLS, :].T).astype(bf)    # [DIM, 512]
        wkT = np.ascontiguousarray(
            wk_p[i * HD:(i + 1) * HD, :].T).astype(bf)          # [DIM, 128]
        wvT = np.ascontiguousarray(
            wv[i * HD:(i + 1) * HD, :].T).astype(bf)
        # out[:, 512i:512(i+1)] = attn_full @ wo.T[:, 512i:...]
        woT = np.ascontiguousarray(
            wo[i * QCOLS:(i + 1) * QCOLS, :].T).astype(bf)      # [DIM, 512]
        wkvP = np.concatenate([_pack_dmajor(wkT), _pack_dmajor(wvT)], axis=1)
        in_maps.append({
            "xT": xT, "wkvP": np.ascontiguousarray(wkvP),
            "wqP": _pack_dmajor(wqT), "woP": _pack_dmajor(woT),
            "ropeC": ropeC, "ropeS": ropeS, "maskmul": maskpack,
        })

    res = run_bass_kernel_spmd(nc, in_maps, list(range(N_CORES)))
    global LAST_RESULT
    LAST_RESULT = res
    out = np.concatenate(
        [np.asarray(res.results[i]["out"]) for i in range(N_CORES)], axis=1)
    return out.reshape(1, SEQ, DIM).astype(np.float32)


# revision 15
# speedup vs baseline: 1.6074x; 1.0784x over previous
"""Tensor-parallel GQA attention layer for 8 Trainium2 NeuronCores.

Shapes (hardcoded from the problem spec):
  x [1, 2048, 4096] f32, wq [4096, 4096], wk/wv [1024, 4096],
  wo [4096, 4096], freqs_cos/sin [2048, 64], mask [2048, 2048].

Sharding: tensor-parallel over heads. Core i owns q-heads 4i..4i+3 and
kv-head i (wq/wk/wv column-parallel). Head outputs are AllGathered per
512-seq chunk (512KB/core) and each core computes out[:, 512i:512(i+1)].

v2 structure (vs v1): single fused QKV pass over x (x streamed once),
static PSUM bank map so no pool-boundary barriers, softmax sums via
vector-chain adds + one f32r matmul per block, reciprocal_approx_fast,
out-projection interleaved chunk-by-chunk behind the AllGathers, DMA
issue in consumption order split between the sync and gpsimd queues.
"""

import math
import sys

for _p in ("/opt/trn_rl_repo",):
    if _p not in sys.path:
        sys.path.append(_p)

import numpy as np
import ml_dtypes

import concourse.bass as bass
import concourse.mybir as mybir
import concourse.tile as tile
from concourse.bass_utils import run_bass_kernel_spmd
from concourse.masks import make_identity
from concourse.vector_clock import ScopedClock

BF16 = mybir.dt.bfloat16
F32 = mybir.dt.float32
F32R = mybir.dt.float32r
AF = mybir.ActivationFunctionType

N_CORES = 8
DIM = 4096
SEQ = 2048
HD = 128                      # head dim == partition dim
NQH = 4                       # q heads per core
P = 128
SC = 512                      # seq chunk (psum bank free size in f32)
ND = DIM // P                 # 32 contraction tiles
NSC = SEQ // SC               # 4 seq chunks
NKT = SEQ // P                # 16 k tiles
QCOLS = NQH * HD              # 512 q columns per core
H = P // 2

LAST_RESULT = None            # BassKernelResults of the most recent kernel() call


def _patch_tile_drain():
    """The walrus build in this container rejects Drain instructions that
    carry more than one sync-wait (and sem-eq waits). Spread the tile-exit
    waits across single-wait nops and use sem-only barriers instead."""

    def patched(self, tick_clock, wait_clock):
        carrier = self.nc.sync.nop(nofuse=True)
        wait_clock.add_sem_waits(
            carrier.ins, ScopedClock({None: tick_clock.global_clock})
        )
        si = carrier.ins.sync_info
        waits = list(si.on_wait) if si and si.on_wait else []
        if len(waits) > 1:
            si.on_wait = waits[:1]
            for w in waits[1:]:
                extra = self.nc.sync.nop(nofuse=True)
                extra.ins.sync_info = mybir.SyncInfo(on_wait=[w], on_update=[])
        self.nc.sync.drain()
        self.nc.all_engine_barrier(sem_only=True)
        popped = self.nc._tile_sem_poison_stack.pop()
        assert popped is self._sem_poison
        self.nc.clear_and_free_semaphores(list(self.sems.allocated().values()))
        self.nc.all_engine_barrier(sem_only=True)

    tile.TileContext._drain_and_barrier = patched


_patch_tile_drain()


def _split_multi_waits(nc, limit=1):
    """This walrus build supports ~one sync-wait per instruction (and none
    on Drain). Hoist excess waits onto single-wait NoOps inserted just
    before the instruction on the same engine queue (FIFO => equivalent)."""
    for fn in nc.m.functions:
        for bb in fn.blocks:
            out = []
            changed = False
            for ins in bb.instructions:
                si = getattr(ins, "sync_info", None)
                waits = list(si.on_wait) if si is not None and si.on_wait else []
                keep = 0 if type(ins).__name__ == "InstDrain" else limit
                if len(waits) > keep:
                    changed = True
                    for w in waits[keep:]:
                        nop = mybir.InstNoOp(
                            name=f"WSPLIT-{nc.next_id()}", ins=[], outs=[])
                        nop.engine = ins.engine
                        nop.sync_info = mybir.SyncInfo(on_wait=[w], on_update=[])
                        out.append(nop)
                    si.on_wait = waits[:keep]
                out.append(ins)
            if changed:
                bb.instructions[:] = out


def _classify_mask(mask):
    """Per (ki, qj) tile classes of exp(mask).T: 0=no-op, 1=multiply, 2=skip.

    Returns (classes [NKT, NSC], packed unique mixed tiles [128, n_u*SC]
    bf16, map {(ki, qj): unique_idx}).
    """
    em = np.exp(mask.astype(np.float64)).astype(np.float32).T  # [k, q]
    classes = np.zeros((NKT, NSC), dtype=np.int32)
    uniq = []
    uniq_keys = {}
    mixed_map = {}
    for ki in range(NKT):
        for qj in range(NSC):
            t = em[ki * P:(ki + 1) * P, qj * SC:(qj + 1) * SC]
            if np.all(t == 1.0):
                classes[ki, qj] = 0
            elif np.all(t == 0.0):
                classes[ki, qj] = 2
            else:
                classes[ki, qj] = 1
                tb = t.astype(ml_dtypes.bfloat16)
                key = tb.tobytes()
                if key not in uniq_keys:
                    uniq_keys[key] = len(uniq)
                    uniq.append(tb)
                mixed_map[(ki, qj)] = uniq_keys[key]
    if uniq:
        packed = np.concatenate(uniq, axis=1)       # [128, n_u*SC]
    else:
        packed = np.zeros((P, SC), dtype=ml_dtypes.bfloat16)
    return classes, packed, mixed_map


def _build_program(classes, mixed_map, n_u):
    nc = bass.Bass()

    xT_d = nc.dram_tensor("xT", [DIM, SEQ], BF16, kind="ExternalInput")
    wkv_d = nc.dram_tensor("wkvP", [P, 2 * ND * HD], BF16, kind="ExternalInput")
    wq_d = nc.dram_tensor("wqP", [P, ND * QCOLS], BF16, kind="ExternalInput")
    wo_d = nc.dram_tensor("woP", [P, ND * QCOLS], BF16, kind="ExternalInput")
    ropeC_d = nc.dram_tensor("ropeC", [P, SEQ], BF16, kind="ExternalInput")
    ropeS_d = nc.dram_tensor("ropeS", [P, SEQ], BF16, kind="ExternalInput")
    mm_d = nc.dram_tensor("maskmul", [P, max(n_u, 1) * SC], BF16,
                          kind="ExternalInput")
    out_d = nc.dram_tensor("out", [SEQ, QCOLS], F32, kind="ExternalOutput")

    scale = 1.0 / math.sqrt(HD)

    with tile.TileContext(nc) as tc, \
         tc.tile_pool(name="const", bufs=1) as cp, \
         tc.tile_pool(name="weights", bufs=1) as wp, \
         tc.tile_pool(name="acts", bufs=1) as ap, \
         tc.tile_pool(name="xw", bufs=3) as xp, \
         tc.tile_pool(name="ropet", bufs=1) as rp, \
         tc.tile_pool(name="esoft", bufs=1) as ep, \
         tc.tile_pool(name="outb", bufs=1) as op, \
         tc.tile_pool(name="ps", bufs=1, space="PSUM") as psp:
        dp = tc.alloc_tile_pool(name="dram", bufs=1, space="DRAM")

        # ---- constants ----
        ident = cp.tile([P, P], BF16, tag="ident", name="ident")
        make_identity(nc, ident[:])
        ones_col = cp.tile([P, 1], BF16, tag="ones_col", name="ones_col")
        nc.gpsimd.memset(ones_col[:], 1.0)
        ones_row = cp.tile([1, P], BF16, tag="ones_row", name="ones_row")
        nc.gpsimd.memset(ones_row[:], 1.0)

        # ---- persistent SBUF tensors ----
        wkv_sb = wp.tile([P, 2 * ND * HD], BF16, tag="wkv", name="wkv_sb")
        wq_sb = wp.tile([P, ND * QCOLS], BF16, tag="wq", name="wq_sb")
        wo_sb = wp.tile([P, ND * QCOLS], BF16, tag="wo", name="wo_sb")
        ropeC = wp.tile([P, SEQ], BF16, tag="ropeC", name="ropeC")
        ropeS = wp.tile([P, SEQ], BF16, tag="ropeS", name="ropeS")
        mask_sb = wp.tile([P, max(n_u, 1) * SC], BF16, tag="mm", name="mask_sb")

        kT = ap.tile([P, SEQ], BF16, tag="kT", name="kT")
        qT = [ap.tile([P, SEQ], BF16, tag=f"qT{h}", name=f"qT{h}")
              for h in range(NQH)]
        V_sb = ap.tile([P, NKT * HD], BF16, tag="V", name="V_sb")
        attnT = [ap.tile([P, SEQ], BF16, tag=f"attnT{h}", name=f"attnT{h}")
                 for h in range(NQH)]

        # ---- DMA issue: weights on sync queue, consumption order ----
        for i in range(8):
            w = 2 * ND * HD // 8
            nc.sync.dma_start(wkv_sb[:, i * w:(i + 1) * w],
                              wkv_d[:, i * w:(i + 1) * w])
        for i in range(4):
            w = SEQ // 4
            nc.sync.dma_start(ropeC[:, i * w:(i + 1) * w],
                              ropeC_d[:, i * w:(i + 1) * w])
            nc.sync.dma_start(ropeS[:, i * w:(i + 1) * w],
                              ropeS_d[:, i * w:(i + 1) * w])
        for i in range(8):
            w = ND * QCOLS // 8
            nc.sync.dma_start(wq_sb[:, i * w:(i + 1) * w],
                              wq_d[:, i * w:(i + 1) * w])
        nc.sync.dma_start(mask_sb[:], mm_d[:])
        for i in range(8):
            w = ND * QCOLS // 8
            nc.sync.dma_start(wo_sb[:, i * w:(i + 1) * w],
                              wo_d[:, i * w:(i + 1) * w])

        # shared DRAM scratch for the collectives
        cc_in = [dp.tile([NQH * P, SC], BF16, tag=f"cc_in{w}",
                         name=f"cc_in{w}") for w in range(NSC)]
        cc_out = [dp.tile([N_CORES * NQH * P, SC], BF16, tag=f"cc_out{w}",
                          name=f"cc_out{w}", addr_space="Shared")
                  for w in range(NSC)]

        # PSUM: 8 static one-bank tags; phase assignments keep overlapping
        # phases on disjoint tags so nothing ever waits on a pool boundary.
        def ps_tile(i, shape, name, dtype=F32):
            return psp.tile(shape, dtype, tag=f"ps{i}", name=name)

        def rope_apply(src, dst, ssl):
            """src: [128, SC] f32 PSUM in deinterleaved-pair layout; writes
            dst[:, ssl] (bf16 SBUF). ropeC=[c;c], ropeS=[-s;s]."""
            tsw = rp.tile([P, SC], F32, tag="tsw", name="tsw")
            nc.scalar.activation(tsw[0:H, :], src[H:P, :], AF.Copy)
            nc.scalar.activation(tsw[H:P, :], src[0:H, :], AF.Copy)
            t1 = rp.tile([P, SC], F32, tag="t1", name="t1")
            nc.vector.tensor_mul(t1[:], src[:], ropeC[:, ssl])
            nc.vector.tensor_mul(tsw[:], tsw[:], ropeS[:, ssl])
            nc.vector.tensor_add(dst[:, ssl], t1[:], tsw[:])

        blk = [0]

        def attn_block(h, qj):
            qsl = slice(qj * SC, (qj + 1) * SC)
            live = [ki for ki in range(NKT) if classes[ki, qj] != 2]
            L = len(live)
            pav = ps_tile(2 + blk[0] % 2, [P, SC], "pav")
            blk[0] += 1
            esum = ep.tile([P, SC], F32, tag="esum", name="esum", bufs=2)
            esb = ep.tile([P, SC], BF16, tag="esb", name="esb", bufs=2)
            for i, ki in enumerate(live):
                pss = ps_tile(i % 2, [P, SC], "pss")
                nc.tensor.matmul(pss[:], kT[:, ki * P:(ki + 1) * P],
                                 qT[h][:, qsl], start=True, stop=True)
                e = ep.tile([P, SC], BF16, tag="e", name="e", bufs=4)
                nc.scalar.activation(e[:], pss[:], AF.Exp, scale=scale)
                u = mixed_map.get((ki, qj))
                if u is not None:
                    nc.vector.tensor_mul(e[:], e[:],
                                         mask_sb[:, u * SC:(u + 1) * SC])
                nc.tensor.matmul(pav[:], V_sb[:, ki * HD:(ki + 1) * HD],
                                 e[:], start=i == 0, stop=i == L - 1)
                # running f32 sum; last add emits the bf16 copy for the MM
                if L == 1:
                    nc.vector.tensor_copy(esb[:], e[:])
                elif i == 0:
                    nc.vector.tensor_copy(esum[:], e[:])
                elif i < L - 1:
                    nc.vector.tensor_add(esum[:], esum[:], e[:])
                else:
                    nc.vector.tensor_add(esb[:], esum[:], e[:])
            sums = ps_tile(4, [1, SC], "sums")
            nc.tensor.matmul(sums[:], ones_col[:], esb[:],
                             start=True, stop=True)
            # 1/s = exp(-ln s) on the scalar engine: [1,512] ops are
            # per-lane serial, and DVE reciprocal costs ~4us there.
            nc.scalar.activation(sums[:], sums[:], AF.Ln)
            rs16 = ep.tile([1, SC], BF16, tag="rs16", name="rs16", bufs=2)
            nc.scalar.activation(rs16[:], sums[:], AF.Exp, scale=-1.0)
            pb = ps_tile(5, [P, SC], "pb")
            nc.tensor.matmul(pb[:], ones_row[:], rs16[:],
                             start=True, stop=True)
            bsb = ep.tile([P, SC], BF16, tag="bsb", name="bsb", bufs=2)
            nc.scalar.activation(bsb[:], pb[:], AF.Copy)
            nc.vector.tensor_mul(attnT[h][:, qsl], pav[:], bsb[:])

        def allgather(qj):
            qsl = slice(qj * SC, (qj + 1) * SC)
            for h in range(NQH):
                nc.gpsimd.dma_start(cc_in[qj][h * P:(h + 1) * P, :],
                                    attnT[h][:, qsl])
            nc.gpsimd.collective_compute(
                "AllGather", mybir.AluOpType.bypass,
                replica_groups=[list(range(N_CORES))],
                ins=[cc_in[qj].opt()], outs=[cc_out[qj].opt()])

        # ---- per-chunk pipeline: QKV(c) -> attention(c) -> AllGather(c).
        # Collectives start early and out-proj (emitted last, lowest
        # priority) fills every later stall.
        for c in range(NSC):
            ssl = slice(c * SC, (c + 1) * SC)
            xts = []
            for do in range(4):
                xt8 = xp.tile([P, 8 * SC], BF16, tag="xt8", name="xt8")
                for j in range(8):
                    d = do * 8 + j
                    nc.gpsimd.dma_start(
                        xt8[:, j * SC:(j + 1) * SC],
                        xT_d[d * P:(d + 1) * P, ssl])
                xts.append(xt8)
            psk = ps_tile(0, [P, SC], "psk")
            psv = ps_tile(1, [P, SC], "psv")
            psq = [ps_tile(2 + h, [P, SC], f"psq{h}") for h in range(NQH)]
            # octet-interleaved KV+Q so an x octet is freed right after its
            # Q matmuls (keeps xt8 residency at 2 live + 1 prefetch)
            for do in range(4):
                for j in range(8):
                    d = do * 8 + j
                    xsl = xts[do][:, j * SC:(j + 1) * SC]
                    st, sp = d == 0, d == ND - 1
                    nc.tensor.matmul(psk[:], wkv_sb[:, d * HD:(d + 1) * HD],
                                     xsl, start=st, stop=sp)
                    nc.tensor.matmul(
                        psv[:],
                        wkv_sb[:, ND * HD + d * HD:ND * HD + (d + 1) * HD],
                        xsl, start=st, stop=sp)
                for j in range(8):
                    d = do * 8 + j
                    xsl = xts[do][:, j * SC:(j + 1) * SC]
                    st, sp = d == 0, d == ND - 1
                    for h in range(NQH):
                        nc.tensor.matmul(
                            psq[h][:],
                            wq_sb[:, d * QCOLS + h * HD:
                                   d * QCOLS + (h + 1) * HD],
                            xsl, start=st, stop=sp)
            rope_apply(psk, kT, ssl)
            vtmp = rp.tile([P, SC], BF16, tag="vtmp", name="vtmp")
            nc.scalar.activation(vtmp[:], psv[:], AF.Copy)
            for h in range(NQH):
                rope_apply(psq[h], qT[h], ssl)
            # V transposes ride banks 2/3 (freed by rope-q0/q1) before the
            # attention pav uses
            for t in range(SC // P):
                kt = c * (SC // P) + t
                ptr = ps_tile(2 + t % 2, [P, P], "ptr", dtype=BF16)
                nc.tensor.transpose(ptr[:], vtmp[:, t * P:(t + 1) * P],
                                    ident[:])
                nc.scalar.activation(V_sb[:, kt * HD:(kt + 1) * HD], ptr,
                                     AF.Copy)
            for h in range(NQH):
                attn_block(h, c)
            allgather(c)

        # ---- out-projection: two s4-pair passes per chunk, ah re-streamed
        def outproj(w):
            NR = N_CORES * NQH
            for pr in range(2):
                po = [ps_tile(6 + s, [P, QCOLS], "po") for s in range(2)]
                for r in range(NR):
                    ah = op.tile([P, SC], BF16, tag="ah", name="ah", bufs=6)
                    nc.sync.dma_start(ah[:], cc_out[w][r * P:(r + 1) * P, :])
                    for s in range(2):
                        s4 = pr * 2 + s
                        nc.tensor.matmul(
                            po[s][:], ah[:, s4 * P:(s4 + 1) * P],
                            wo_sb[:, r * SC:(r + 1) * SC],
                            start=r == 0, stop=r == NR - 1)
                for s in range(2):
                    s4 = pr * 2 + s
                    ob = op.tile([P, QCOLS], F32, tag="ob", name="ob", bufs=2)
                    nc.scalar.activation(ob[:], po[s][:], AF.Copy)
                    st = w * (SC // P) + s4
                    nc.sync.dma_start(out_d[st * P:(st + 1) * P, :], ob[:])

        for w in range(NSC):
            outproj(w)
        dp.release()

    _split_multi_waits(nc)
    return nc


def _pack_dmajor(wT):
    """[DIM, W] -> [128, ND*W] with d-tile d at cols [d*W, (d+1)*W)."""
    w = wT.shape[1]
    return np.ascontiguousarray(
        wT.reshape(ND, P, w).transpose(1, 0, 2).reshape(P, ND * w))


def kernel(x, wq, wk, wv, wo, freqs_cos, freqs_sin, mask):
    x = np.asarray(x, dtype=np.float32)
    wq = np.asarray(wq, dtype=np.float32)
    wk = np.asarray(wk, dtype=np.float32)
    wv = np.asarray(wv, dtype=np.float32)
    wo = np.asarray(wo, dtype=np.float32)
    freqs_cos = np.asarray(freqs_cos, dtype=np.float32)
    freqs_sin = np.asarray(freqs_sin, dtype=np.float32)
    mask = np.asarray(mask, dtype=np.float32)

    bf = ml_dtypes.bfloat16
    # deinterleave head_dim pairs so RoPE becomes a partition-half swap
    perm = np.concatenate([np.arange(0, HD, 2), np.arange(1, HD, 2)])
    wq_p = wq.reshape(-1, HD, DIM)[:, perm, :].reshape(wq.shape)
    wk_p = wk.reshape(-1, HD, DIM)[:, perm, :].reshape(wk.shape)

    xT = np.ascontiguousarray(x[0].T).astype(bf)               # [DIM, SEQ]
    ropeC = np.ascontiguousarray(
        np.concatenate([freqs_cos.T, freqs_cos.T], axis=0)).astype(bf)
    ropeS = np.ascontiguousarray(
        np.concatenate([-freqs_sin.T, freqs_sin.T], axis=0)).astype(bf)

    classes, maskpack, mixed_map = _classify_mask(mask)
    n_u = maskpack.shape[1] // SC if mixed_map else 0

    nc = _build_program(classes, mixed_map, n_u)

    in_maps = []
    for i in range(N_CORES):
        wqT = np.ascontiguousarray(
            wq_p[i * QCOLS:(i + 1) * QCOLS, :].T).astype(bf)    # [DIM, 512]
        wkT = np.ascontiguousarray(
            wk_p[i * HD:(i + 1) * HD, :].T).astype(bf)          # [DIM, 128]
        wvT = np.ascontiguousarray(
            wv[i * HD:(i + 1) * HD, :].T).astype(bf)
        # out[:, 512i:512(i+1)] = attn_full @ wo.T[:, 512i:...]
        woT = np.ascontiguousarray(
            wo[i * QCOLS:(i + 1) * QCOLS, :].T).astype(bf)      # [DIM, 512]
        wkvP = np.concatenate([_pack_dmajor(wkT), _pack_dmajor(wvT)], axis=1)
        in_maps.append({
            "xT": xT, "wkvP": np.ascontiguousarray(wkvP),
            "wqP": _pack_dmajor(wqT), "woP": _pack_dmajor(woT),
            "ropeC": ropeC, "ropeS": ropeS, "maskmul": maskpack,
        })

    res = run_bass_kernel_spmd(nc, in_maps, list(range(N_CORES)))
    global LAST_RESULT
    LAST_RESULT = res
    out = np.concatenate(
        [np.asarray(res.results[i]["out"]) for i in range(N_CORES)], axis=1)
    return out.reshape(1, SEQ, DIM).astype(np.float32)


# revision 19
# speedup vs baseline: 1.6516x; 1.0275x over previous
"""Tensor-parallel GQA attention layer for 8 Trainium2 NeuronCores.

Shapes (hardcoded from the problem spec):
  x [1, 2048, 4096] f32, wq [4096, 4096], wk/wv [1024, 4096],
  wo [4096, 4096], freqs_cos/sin [2048, 64], mask [2048, 2048].

Sharding: tensor-parallel over heads. Core i owns q-heads 4i..4i+3 and
kv-head i (wq/wk/wv column-parallel). Head outputs are AllGathered per
512-seq chunk (512KB/core) and each core computes out[:, 512i:512(i+1)].

v2 structure (vs v1): single fused QKV pass over x (x streamed once),
static PSUM bank map so no pool-boundary barriers, softmax sums via
vector-chain adds + one f32r matmul per block, reciprocal_approx_fast,
out-projection interleaved chunk-by-chunk behind the AllGathers, DMA
issue in consumption order split between the sync and gpsimd queues.
"""

import math
import sys

for _p in ("/opt/trn_rl_repo",):
    if _p not in sys.path:
        sys.path.append(_p)

import numpy as np
import ml_dtypes

import concourse.bass as bass
import concourse.mybir as mybir
import concourse.tile as tile
from concourse.bass_utils import run_bass_kernel_spmd
from concourse.masks import make_identity
from concourse.vector_clock import ScopedClock

BF16 = mybir.dt.bfloat16
F32 = mybir.dt.float32
F32R = mybir.dt.float32r
AF = mybir.ActivationFunctionType

N_CORES = 8
DIM = 4096
SEQ = 2048
HD = 128                      # head dim == partition dim
NQH = 4                       # q heads per core
P = 128
SC = 512                      # seq chunk (psum bank free size in f32)
ND = DIM // P                 # 32 contraction tiles
NSC = SEQ // SC               # 4 seq chunks
NKT = SEQ // P                # 16 k tiles
QCOLS = NQH * HD              # 512 q columns per core
H = P // 2

LAST_RESULT = None            # BassKernelResults of the most recent kernel() call


def _patch_tile_drain():
    """The walrus build in this container rejects Drain instructions that
    carry more than one sync-wait (and sem-eq waits). Spread the tile-exit
    waits across single-wait nops and use sem-only barriers instead."""

    def patched(self, tick_clock, wait_clock):
        carrier = self.nc.sync.nop(nofuse=True)
        wait_clock.add_sem_waits(
            carrier.ins, ScopedClock({None: tick_clock.global_clock})
        )
        si = carrier.ins.sync_info
        waits = list(si.on_wait) if si and si.on_wait else []
        if len(waits) > 1:
            si.on_wait = waits[:1]
            for w in waits[1:]:
                extra = self.nc.sync.nop(nofuse=True)
                extra.ins.sync_info = mybir.SyncInfo(on_wait=[w], on_update=[])
        self.nc.sync.drain()
        self.nc.all_engine_barrier(sem_only=True)
        popped = self.nc._tile_sem_poison_stack.pop()
        assert popped is self._sem_poison
        self.nc.clear_and_free_semaphores(list(self.sems.allocated().values()))
        self.nc.all_engine_barrier(sem_only=True)

    tile.TileContext._drain_and_barrier = patched


_patch_tile_drain()


def _split_multi_waits(nc, limit=1):
    """This walrus build supports ~one sync-wait per instruction (and none
    on Drain). Hoist excess waits onto single-wait NoOps inserted just
    before the instruction on the same engine queue (FIFO => equivalent)."""
    for fn in nc.m.functions:
        for bb in fn.blocks:
            out = []
            changed = False
            for ins in bb.instructions:
                si = getattr(ins, "sync_info", None)
                waits = list(si.on_wait) if si is not None and si.on_wait else []
                keep = 0 if type(ins).__name__ == "InstDrain" else limit
                if len(waits) > keep:
                    changed = True
                    for w in waits[keep:]:
                        nop = mybir.InstNoOp(
                            name=f"WSPLIT-{nc.next_id()}", ins=[], outs=[])
                        nop.engine = ins.engine
                        nop.sync_info = mybir.SyncInfo(on_wait=[w], on_update=[])
                        out.append(nop)
                    si.on_wait = waits[:keep]
                out.append(ins)
            if changed:
                bb.instructions[:] = out


def _classify_mask(mask):
    """Per (ki, qj) tile classes of exp(mask).T: 0=no-op, 1=multiply, 2=skip.

    Returns (classes [NKT, NSC], packed unique mixed tiles [128, n_u*SC]
    bf16, map {(ki, qj): unique_idx}).
    """
    em = np.exp(mask.astype(np.float64)).astype(np.float32).T  # [k, q]
    classes = np.zeros((NKT, NSC), dtype=np.int32)
    uniq = []
    uniq_keys = {}
    mixed_map = {}
    for ki in range(NKT):
        for qj in range(NSC):
            t = em[ki * P:(ki + 1) * P, qj * SC:(qj + 1) * SC]
            if np.all(t == 1.0):
                classes[ki, qj] = 0
            elif np.all(t == 0.0):
                classes[ki, qj] = 2
            else:
                classes[ki, qj] = 1
                tb = t.astype(ml_dtypes.bfloat16)
                key = tb.tobytes()
                if key not in uniq_keys:
                    uniq_keys[key] = len(uniq)
                    uniq.append(tb)
                mixed_map[(ki, qj)] = uniq_keys[key]
    if uniq:
        packed = np.concatenate(uniq, axis=1)       # [128, n_u*SC]
    else:
        packed = np.zeros((P, SC), dtype=ml_dtypes.bfloat16)
    return classes, packed, mixed_map


def _build_program(classes, mixed_map, n_u):
    nc = bass.Bass()

    xT_d = nc.dram_tensor("xT", [DIM, SEQ], BF16, kind="ExternalInput")
    wkv_d = nc.dram_tensor("wkvP", [P, 2 * ND * HD], BF16, kind="ExternalInput")
    wq_d = nc.dram_tensor("wqP", [P, ND * QCOLS], BF16, kind="ExternalInput")
    wo_d = nc.dram_tensor("woP", [P, ND * QCOLS], BF16, kind="ExternalInput")
    ropeC_d = nc.dram_tensor("ropeC", [P, SEQ], BF16, kind="ExternalInput")
    ropeS_d = nc.dram_tensor("ropeS", [P, SEQ], BF16, kind="ExternalInput")
    mm_d = nc.dram_tensor("maskmul", [P, max(n_u, 1) * SC], BF16,
                          kind="ExternalInput")
    out_d = nc.dram_tensor("out", [SEQ, QCOLS], F32, kind="ExternalOutput")

    scale = 1.0 / math.sqrt(HD)

    with tile.TileContext(nc) as tc, \
         tc.tile_pool(name="const", bufs=1) as cp, \
         tc.tile_pool(name="weights", bufs=1) as wp, \
         tc.tile_pool(name="acts", bufs=1) as ap, \
         tc.tile_pool(name="xw", bufs=4) as xp, \
         tc.tile_pool(name="ropet", bufs=1) as rp, \
         tc.tile_pool(name="esoft", bufs=1) as ep, \
         tc.tile_pool(name="outb", bufs=1) as op, \
         tc.tile_pool(name="ps", bufs=1, space="PSUM") as psp:
        dp = tc.alloc_tile_pool(name="dram", bufs=1, space="DRAM")

        # ---- constants ----
        ident = cp.tile([P, P], BF16, tag="ident", name="ident")
        make_identity(nc, ident[:])
        ones_col = cp.tile([P, 1], BF16, tag="ones_col", name="ones_col")
        nc.vector.memset(ones_col[:], 1.0)
        ones_row = cp.tile([1, P], BF16, tag="ones_row", name="ones_row")
        nc.vector.memset(ones_row[:], 1.0)

        # ---- persistent SBUF tensors ----
        wkv_sb = wp.tile([P, 2 * ND * HD], BF16, tag="wkv", name="wkv_sb")
        wq_sb = wp.tile([P, ND * QCOLS], BF16, tag="wq", name="wq_sb")
        wo_sb = wp.tile([P, ND * QCOLS], BF16, tag="wo", name="wo_sb")
        ropeC = wp.tile([P, SEQ], BF16, tag="ropeC", name="ropeC")
        ropeS = wp.tile([P, SEQ], BF16, tag="ropeS", name="ropeS")
        mask_sb = wp.tile([P, max(n_u, 1) * SC], BF16, tag="mm", name="mask_sb")

        kT = ap.tile([P, SEQ], BF16, tag="kT", name="kT")
        qT = [ap.tile([P, SEQ], BF16, tag=f"qT{h}", name=f"qT{h}")
              for h in range(NQH)]
        V_sb = ap.tile([P, NKT * HD], BF16, tag="V", name="V_sb")
        attnT = [ap.tile([P, SEQ], BF16, tag=f"attnT{h}", name=f"attnT{h}")
                 for h in range(NQH)]

        # ---- DMA issue: weights on sync queue, consumption order ----
        for i in range(16):
            w = 2 * ND * HD // 16
            nc.sync.dma_start(wkv_sb[:, i * w:(i + 1) * w],
                              wkv_d[:, i * w:(i + 1) * w])
        for i in range(4):
            w = SEQ // 4
            nc.sync.dma_start(ropeC[:, i * w:(i + 1) * w],
                              ropeC_d[:, i * w:(i + 1) * w])
            nc.sync.dma_start(ropeS[:, i * w:(i + 1) * w],
                              ropeS_d[:, i * w:(i + 1) * w])
        for i in range(8):
            w = ND * QCOLS // 8
            nc.sync.dma_start(wq_sb[:, i * w:(i + 1) * w],
                              wq_d[:, i * w:(i + 1) * w])
        nc.sync.dma_start(mask_sb[:], mm_d[:])
        for i in range(8):
            w = ND * QCOLS // 8
            nc.sync.dma_start(wo_sb[:, i * w:(i + 1) * w],
                              wo_d[:, i * w:(i + 1) * w])

        # shared DRAM scratch for the collectives
        cc_in = [dp.tile([NQH * P, SC], BF16, tag=f"cc_in{w}",
                         name=f"cc_in{w}") for w in range(NSC)]
        cc_out = [dp.tile([N_CORES * NQH * P, SC], BF16, tag=f"cc_out{w}",
                          name=f"cc_out{w}", addr_space="Shared")
                  for w in range(NSC)]

        # PSUM: 8 static one-bank tags; phase assignments keep overlapping
        # phases on disjoint tags so nothing ever waits on a pool boundary.
        def ps_tile(i, shape, name, dtype=F32):
            return psp.tile(shape, dtype, tag=f"ps{i}", name=name)

        def rope_apply(src, dst, ssl):
            """src: [128, SC] f32 PSUM in deinterleaved-pair layout; writes
            dst[:, ssl] (bf16 SBUF). ropeC=[c;c], ropeS=[-s;s]."""
            tsw = rp.tile([P, SC], F32, tag="tsw", name="tsw")
            nc.scalar.activation(tsw[0:H, :], src[H:P, :], AF.Copy)
            nc.scalar.activation(tsw[H:P, :], src[0:H, :], AF.Copy)
            t1 = rp.tile([P, SC], F32, tag="t1", name="t1")
            nc.vector.tensor_mul(t1[:], src[:], ropeC[:, ssl])
            nc.vector.tensor_mul(tsw[:], tsw[:], ropeS[:, ssl])
            nc.vector.tensor_add(dst[:, ssl], t1[:], tsw[:])

        blk = [0]

        def attn_block(h, qj):
            qsl = slice(qj * SC, (qj + 1) * SC)
            live = [ki for ki in range(NKT) if classes[ki, qj] != 2]
            L = len(live)
            pav = ps_tile(2 + blk[0] % 2, [P, SC], "pav")
            blk[0] += 1
            esum = ep.tile([P, SC], F32, tag="esum", name="esum", bufs=2)
            esb = ep.tile([P, SC], BF16, tag="esb", name="esb", bufs=2)
            for i, ki in enumerate(live):
                pss = ps_tile(i % 2, [P, SC], "pss")
                nc.tensor.matmul(pss[:], kT[:, ki * P:(ki + 1) * P],
                                 qT[h][:, qsl], start=True, stop=True)
                e = ep.tile([P, SC], BF16, tag="e", name="e", bufs=4)
                nc.scalar.activation(e[:], pss[:], AF.Exp, scale=scale)
                u = mixed_map.get((ki, qj))
                if u is not None:
                    nc.vector.tensor_mul(e[:], e[:],
                                         mask_sb[:, u * SC:(u + 1) * SC])
                nc.tensor.matmul(pav[:], V_sb[:, ki * HD:(ki + 1) * HD],
                                 e[:], start=i == 0, stop=i == L - 1)
                # running f32 sum; last add emits the bf16 copy for the MM
                if L == 1:
                    nc.vector.tensor_copy(esb[:], e[:])
                elif i == 0:
                    nc.vector.tensor_copy(esum[:], e[:])
                elif i < L - 1:
                    nc.vector.tensor_add(esum[:], esum[:], e[:])
                else:
                    nc.vector.tensor_add(esb[:], esum[:], e[:])
            sums = ps_tile(4, [1, SC], "sums")
            nc.tensor.matmul(sums[:], ones_col[:], esb[:],
                             start=True, stop=True)
            # 1/s = exp(-ln s) on the scalar engine: [1,512] ops are
            # per-lane serial, and DVE reciprocal costs ~4us there.
            nc.scalar.activation(sums[:], sums[:], AF.Ln)
            rs16 = ep.tile([1, SC], BF16, tag="rs16", name="rs16", bufs=2)
            nc.scalar.activation(rs16[:], sums[:], AF.Exp, scale=-1.0)
            pb = ps_tile(5, [P, SC], "pb")
            nc.tensor.matmul(pb[:], ones_row[:], rs16[:],
                             start=True, stop=True)
            bsb = ep.tile([P, SC], BF16, tag="bsb", name="bsb", bufs=2)
            nc.scalar.activation(bsb[:], pb[:], AF.Copy)
            nc.vector.tensor_mul(attnT[h][:, qsl], pav[:], bsb[:])

        def allgather(qj):
            qsl = slice(qj * SC, (qj + 1) * SC)
            for h in range(NQH):
                nc.gpsimd.dma_start(cc_in[qj][h * P:(h + 1) * P, :],
                                    attnT[h][:, qsl])
            nc.gpsimd.collective_compute(
                "AllGather", mybir.AluOpType.bypass,
                replica_groups=[list(range(N_CORES))],
                ins=[cc_in[qj].opt()], outs=[cc_out[qj].opt()])

        # ---- per-chunk pipeline: QKV(c) -> attention(c) -> AllGather(c).
        # Collectives start early and out-proj (emitted last, lowest
        # priority) fills every later stall.
        for c in range(NSC):
            ssl = slice(c * SC, (c + 1) * SC)
            xts = []
            for do in range(4):
                xt8 = xp.tile([P, 8 * SC], BF16, tag="xt8", name="xt8")
                for j in range(8):
                    d = do * 8 + j
                    nc.gpsimd.dma_start(
                        xt8[:, j * SC:(j + 1) * SC],
                        xT_d[d * P:(d + 1) * P, ssl])
                xts.append(xt8)
            psk = ps_tile(0, [P, SC], "psk")
            psv = ps_tile(1, [P, SC], "psv")
            psq = [ps_tile(2 + h, [P, SC], f"psq{h}") for h in range(NQH)]
            # octet-interleaved KV+Q so an x octet is freed right after its
            # Q matmuls (keeps xt8 residency at 2 live + 1 prefetch)
            for do in range(4):
                for j in range(8):
                    d = do * 8 + j
                    xsl = xts[do][:, j * SC:(j + 1) * SC]
                    st, sp = d == 0, d == ND - 1
                    nc.tensor.matmul(psk[:], wkv_sb[:, d * HD:(d + 1) * HD],
                                     xsl, start=st, stop=sp)
                    nc.tensor.matmul(
                        psv[:],
                        wkv_sb[:, ND * HD + d * HD:ND * HD + (d + 1) * HD],
                        xsl, start=st, stop=sp)
                for j in range(8):
                    d = do * 8 + j
                    xsl = xts[do][:, j * SC:(j + 1) * SC]
                    st, sp = d == 0, d == ND - 1
                    for h in range(NQH):
                        nc.tensor.matmul(
                            psq[h][:],
                            wq_sb[:, d * QCOLS + h * HD:
                                   d * QCOLS + (h + 1) * HD],
                            xsl, start=st, stop=sp)
            rope_apply(psk, kT, ssl)
            vtmp = rp.tile([P, SC], BF16, tag="vtmp", name="vtmp")
            nc.scalar.activation(vtmp[:], psv[:], AF.Copy)
            for h in range(NQH):
                rope_apply(psq[h], qT[h], ssl)
            # V transposes ride banks 2/3 (freed by rope-q0/q1) before the
            # attention pav uses
            for t in range(SC // P):
                kt = c * (SC // P) + t
                ptr = ps_tile(2 + t % 2, [P, P], "ptr", dtype=BF16)
                nc.tensor.transpose(ptr[:], vtmp[:, t * P:(t + 1) * P],
                                    ident[:])
                nc.scalar.activation(V_sb[:, kt * HD:(kt + 1) * HD], ptr,
                                     AF.Copy)
            for h in range(NQH):
                attn_block(h, c)
            allgather(c)

        # ---- out-projection. Early chunks run as low-priority filler during
        # the remaining attention (only banks 6/7 free -> two s4-pair passes,
        # ah streamed twice). Late chunks run after attention ends, so banks
        # 4-7 are free -> single ah stream, 4 accumulators.
        def outproj(w, wide):
            NR = N_CORES * NQH
            if wide:
                po = [ps_tile(4 + s4, [P, QCOLS], "po") for s4 in range(4)]
                for r in range(NR):
                    ah = op.tile([P, SC], BF16, tag="ah", name="ah", bufs=6)
                    nc.sync.dma_start(ah[:], cc_out[w][r * P:(r + 1) * P, :])
                    for s4 in range(4):
                        nc.tensor.matmul(
                            po[s4][:], ah[:, s4 * P:(s4 + 1) * P],
                            wo_sb[:, r * SC:(r + 1) * SC],
                            start=r == 0, stop=r == NR - 1)
                for s4 in range(4):
                    ob = op.tile([P, QCOLS], F32, tag="ob", name="ob", bufs=2)
                    nc.scalar.activation(ob[:], po[s4][:], AF.Copy)
                    st = w * (SC // P) + s4
                    nc.sync.dma_start(out_d[st * P:(st + 1) * P, :], ob[:])
                return
            for pr in range(2):
                po = [ps_tile(6 + s, [P, QCOLS], "po") for s in range(2)]
                for r in range(NR):
                    ah = op.tile([P, SC], BF16, tag="ah", name="ah", bufs=6)
                    nc.sync.dma_start(ah[:], cc_out[w][r * P:(r + 1) * P, :])
                    for s in range(2):
                        s4 = pr * 2 + s
                        nc.tensor.matmul(
                            po[s][:], ah[:, s4 * P:(s4 + 1) * P],
                            wo_sb[:, r * SC:(r + 1) * SC],
                            start=r == 0, stop=r == NR - 1)
                for s in range(2):
                    s4 = pr * 2 + s
                    ob = op.tile([P, QCOLS], F32, tag="ob", name="ob", bufs=2)
                    nc.scalar.activation(ob[:], po[s][:], AF.Copy)
                    st = w * (SC // P) + s4
                    nc.sync.dma_start(out_d[st * P:(st + 1) * P, :], ob[:])

        for w in range(NSC):
            outproj(w, wide=w >= 2)
        dp.release()

    _split_multi_waits(nc)
    return nc


def _pack_dmajor(wT):
    """[DIM, W] -> [128, ND*W] with d-tile d at cols [d*W, (d+1)*W)."""
    w = wT.shape[1]
    return np.ascontiguousarray(
        wT.reshape(ND, P, w).transpose(1, 0, 2).reshape(P, ND * w))


def kernel(x, wq, wk, wv, wo, freqs_cos, freqs_sin, mask):
    x = np.asarray(x, dtype=np.float32)
    wq = np.asarray(wq, dtype=np.float32)
    wk = np.asarray(wk, dtype=np.float32)
    wv = np.asarray(wv, dtype=np.float32)
    wo = np.asarray(wo, dtype=np.float32)
    freqs_cos = np.asarray(freqs_cos, dtype=np.float32)
    freqs_sin = np.asarray(freqs_sin, dtype=np.float32)
    mask = np.asarray(mask, dtype=np.float32)

    bf = ml_dtypes.bfloat16
    # deinterleave head_dim pairs so RoPE becomes a partition-half swap
    perm = np.concatenate([np.arange(0, HD, 2), np.arange(1, HD, 2)])
    wq_p = wq.reshape(-1, HD, DIM)[:, perm, :].reshape(wq.shape)
    wk_p = wk.reshape(-1, HD, DIM)[:, perm, :].reshape(wk.shape)

    xT = np.ascontiguousarray(x[0].T).astype(bf)               # [DIM, SEQ]
    ropeC = np.ascontiguousarray(
        np.concatenate([freqs_cos.T, freqs_cos.T], axis=0)).astype(bf)
    ropeS = np.ascontiguousarray(
        np.concatenate([-freqs_sin.T, freqs_sin.T], axis=0)).astype(bf)

    classes, maskpack, mixed_map = _classify_mask(mask)
    n_u = maskpack.shape[1] // SC if mixed_map else 0

    nc = _build_program(classes, mixed_map, n_u)

    in_maps = []
    for i in range(N_CORES):
        wqT = np.ascontiguousarray(
            wq_p[i * QCOLS:(i + 1) * QCOLS, :].T).astype(bf)    # [DIM, 512]
        wkT = np.ascontiguousarray(
            wk_p[i * HD:(i + 1) * HD, :].T).astype(bf)          # [DIM, 128]
        wvT = np.ascontiguousarray(
            wv[i * HD:(i + 1) * HD, :].T).astype(bf)
        # out[:, 512i:512(i+1)] = attn_full @ wo.T[:, 512i:...]
        woT = np.ascontiguousarray(
            wo[i * QCOLS:(i + 1) * QCOLS, :].T).astype(bf)      # [DIM, 512]
        wkvP = np.concatenate([_pack_dmajor(wkT), _pack_dmajor(wvT)], axis=1)
        in_maps.append({
            "xT": xT, "wkvP": np.ascontiguousarray(wkvP),
            "wqP": _pack_dmajor(wqT), "woP": _pack_dmajor(woT),
            "ropeC": ropeC, "ropeS": ropeS, "maskmul": maskpack,
        })

    res = run_bass_kernel_spmd(nc, in_maps, list(range(N_CORES)))
    global LAST_RESULT
    LAST_RESULT = res
    out = np.concatenate(
        [np.asarray(res.results[i]["out"]) for i in range(N_CORES)], axis=1)
    return out.reshape(1, SEQ, DIM).astype(np.float32)
